# revision 32
# baseline (speedup 1.0000x reference)
"""Additive (Bahdanau) attention TRN2 Bass kernel.

Problem (hardcoded shapes):
    query (4, 512, 256), key (4, 512, 256), value (4, 512, 256)
    W_q (256, 256), W_k (256, 256), W_v (256,)
    q = query @ W_q ; k = key @ W_k
    scores[b,n,m] = sum_h W_v[h] * tanh(q[b,n,h] + k[b,m,h])
    out = softmax_m(scores) @ value          -> (4, 512, 256)

Sharding: 8 cores, data-parallel over (batch, query-half):
    core c handles batch b = c // 2, query rows [ (c%2)*256, (c%2)*256+256 ).
Each core sees the full key/value of its batch; outputs are disjoint row
blocks of the full output, so no collectives are needed.

Per-core device algorithm (all fp32):
  1. PE-transpose query/key chunks, project with W_q/W_k -> qT (h,n), kT (h,m)
     laid out with h on partitions.
  2. For each query row n: DVE tensor_scalar add broadcasts q_n[h] over the
     kT tile (h, m); groups of 8 rows share one big ACT tanh instruction.
  3. The h-reduction with W_v runs on PE as a matmul whose stationary is a
     sliding 128-column window of a zero-padded W_v buffer (a delta-column
     matrix), accumulating each row's scores into its own PSUM partition.
  4. Softmax skips the max-subtraction (scores are O(1) by construction):
     ACT exp emits e and its row sums in one instruction, DVE takes
     reciprocals, and the normalization is folded into the output rows.
  5. e is PE-transposed and attn @ value runs as 4 accumulating matmuls.
"""

import os
import time

import numpy as np

N, NQ, M, DQ, DK, DV, H = 4, 512, 512, 256, 256, 256, 256
NCORES = 8
NQC = N * NQ // NCORES  # query rows per core = 256
GROUP = 8  # query rows per ACT tanh instruction
NBLK = NQC // 128  # PSUM score blocks per core

_runner = None


def _build_program():
    from contextlib import ExitStack

    import concourse.bass as bass
    import concourse.mybir as mybir
    import concourse.tile as tile
    from concourse.masks import make_identity
    from concourse.vector_clock import ScopedClock

    f32 = mybir.dt.float32
    bf16 = mybir.dt.bfloat16
    AF = mybir.ActivationFunctionType

    class TileContextChunkedDrain(tile.TileContext):
        """This walrus build rejects instructions carrying more than one sync
        wait. Tile's scheduler freely attaches several, both on scheduled
        instructions and on the exit drain — hoist the extras onto
        single-wait NOPs on the same engine."""

        def _lower_ordered_insts(self, ordered):
            for bb_name, insts in ordered.items():
                new = []
                for inst in insts:
                    si = inst.sync_info
                    if si is not None and si.on_wait and len(si.on_wait) > 1:
                        waits = list(si.on_wait)
                        for wi, w in enumerate(waits[:-1]):
                            nop = mybir.InstNoOp(
                                name=f"{inst.name}-sw{wi}", ins=[], outs=[]
                            )
                            nop.engine = inst.engine
                            nop.sync_info = mybir.SyncInfo(
                                on_wait=[w], on_update=[]
                            )
                            new.append(nop)
                        inst.sync_info = mybir.SyncInfo(
                            on_wait=[waits[-1]], on_update=list(si.on_update)
                        )
                    new.append(inst)
                ordered[bb_name] = new
            return super()._lower_ordered_insts(ordered)

        def _drain_and_barrier(self, tick_clock, wait_clock):
            nc = self.nc
            probe = nc.sync.nop(nofuse=True)
            wait_clock.add_sem_waits(
                probe.ins, ScopedClock({None: tick_clock.global_clock})
            )
            waits = list(probe.ins.sync_info.on_wait)
            probe.ins.sync_info = mybir.SyncInfo(on_wait=waits[:1], on_update=[])
            for w in waits[1:]:
                n2 = nc.sync.nop(nofuse=True)
                n2.ins.sync_info = mybir.SyncInfo(on_wait=[w], on_update=[])
            nc.sync.drain()
            nc.all_engine_barrier()
            popped = nc._tile_sem_poison_stack.pop()
            assert popped is self._sem_poison
            nc.clear_and_free_semaphores(list(self.sems.allocated().values()))
            nc.all_engine_barrier()

    nc = bass.Bass(enable_partition_id=False)
    q_ext = nc.dram_tensor("query", [NQC, DQ], f32, kind="ExternalInput")
    k_ext = nc.dram_tensor("key", [M, DK], f32, kind="ExternalInput")
    v_ext = nc.dram_tensor("value", [M, DV], f32, kind="ExternalInput")
    wq_ext = nc.dram_tensor("W_q", [DQ, H], f32, kind="ExternalInput")
    wk_ext = nc.dram_tensor("W_k", [DK, H], f32, kind="ExternalInput")
    wv_ext = nc.dram_tensor("W_v", [H, 1], f32, kind="ExternalInput")
    out_ext = nc.dram_tensor("out", [NQC, DV], f32, kind="ExternalOutput")

    with TileContextChunkedDrain(nc) as tc, ExitStack() as ctx:
        singles = ctx.enter_context(tc.tile_pool(name="singles", bufs=1))
        loads = ctx.enter_context(tc.tile_pool(name="loads", bufs=2))
        work_pool = ctx.enter_context(tc.tile_pool(name="work", bufs=4))
        e_pool = ctx.enter_context(tc.tile_pool(name="epool", bufs=2))
        et_pool = ctx.enter_context(tc.tile_pool(name="etpool", bufs=2))
        out_pool = ctx.enter_context(tc.tile_pool(name="outpool", bufs=2))
        small = ctx.enter_context(tc.tile_pool(name="small", bufs=4))
        early_ctx = ExitStack()
        ps_early = early_ctx.enter_context(
            tc.tile_pool(name="ps_early", bufs=2, space="PSUM")
        )

        # ---- constants ----
        identity = singles.tile([128, 128], f32)
        make_identity(nc, identity)

        # ---- coalesced input DMAs, critical (key) path first ----
        knat_t = loads.tile([128, 4, DK], f32, name="knat_t")
        nc.sync.dma_start(out=knat_t, in_=k_ext.rearrange("(c p) d -> p c d", p=128))
        wk_s = singles.tile([128, 2, H], f32)
        nc.sync.dma_start(out=wk_s, in_=wk_ext.rearrange("(c p) h -> p c h", p=128))
        qnat_t = loads.tile([128, 2, DQ], f32, name="qnat_t")
        nc.sync.dma_start(out=qnat_t, in_=q_ext.rearrange("(c p) d -> p c d", p=128))
        wq_s = singles.tile([128, 2, H], f32)
        nc.sync.dma_start(out=wq_s, in_=wq_ext.rearrange("(c p) h -> p c h", p=128))
        wv_f = singles.tile([128, 2], f32)
        nc.sync.dma_start(out=wv_f, in_=wv_ext.rearrange("(c p) one -> p (c one)", p=128))
        value_s = singles.tile([128, 4, DV], f32)
        nc.sync.dma_start(out=value_s, in_=v_ext.rearrange("(c p) d -> p c d", p=128))

        # z2[:, x, :] = [0]*128 ++ [W_v[x*128:(x+1)*128]] ++ [0]*127 ; the
        # 128-column window starting at 128-s is W_v * delta(col == s).
        # bf16 so the scores matmul is a single-pass bf16 matmul (fp32
        # matmuls lower to two PE passes).
        z2 = singles.tile([128, 2, 256], bf16)
        nc.vector.memset(z2, 0.0)
        for x in range(2):
            nc.vector.tensor_copy(z2[:, x, 128:129], wv_f[:, x : x + 1])

        qnat = [qnat_t[:, i, :] for i in range(2)]
        knat = [knat_t[:, i, :] for i in range(4)]

        qTd = singles.tile([128, 2, NQC], f32)  # (d_local, dc, n)
        for dc in range(2):
            ps = ps_early.tile([128, 512], f32, name="ps")
            for nck in range(2):
                nc.tensor.transpose(
                    ps[:, nck * 128 : (nck + 1) * 128],
                    qnat[nck][:, dc * 128 : (dc + 1) * 128],
                    identity,
                )
            nc.vector.tensor_copy(qTd[:, dc, :], ps[:, :NQC])

        kTd = singles.tile([128, 2, M], f32)  # (d_local, dc, m)
        for dc in range(2):
            ps = ps_early.tile([128, 512], f32, name="ps")
            for mck in range(4):
                nc.tensor.transpose(
                    ps[:, mck * 128 : (mck + 1) * 128],
                    knat[mck][:, dc * 128 : (dc + 1) * 128],
                    identity,
                )
            nc.scalar.copy(kTd[:, dc, :], ps)

        # ---- projections: qT (h, n), kT (h, m) ----
        # kT is bf16 so the DVE broadcast-adds (bf16 in0 + bf16 out, fp32
        # per-partition scalar) hit the 4x_2p perf mode.
        qT = [singles.tile([128, NQC], f32, name=f"qT{hc}") for hc in range(2)]
        kT = [singles.tile([128, M], bf16, name=f"kT{hc}") for hc in range(2)]
        for hc in range(2):
            ps = ps_early.tile([128, 512], f32, name="ps")
            for dc in range(2):
                nc.tensor.matmul(
                    ps,
                    lhsT=wk_s[:, dc, hc * 128 : (hc + 1) * 128],
                    rhs=kTd[:, dc, :],
                    start=(dc == 0),
                    stop=(dc == 1),
                )
            nc.scalar.copy(kT[hc], ps)
        for hc in range(2):
            ps = ps_early.tile([128, 512], f32, name="ps")
            for dc in range(2):
                nc.tensor.matmul(
                    ps[:, :NQC],
                    lhsT=wq_s[:, dc, hc * 128 : (hc + 1) * 128],
                    rhs=qTd[:, dc, :],
                    start=(dc == 0),
                    stop=(dc == 1),
                )
            nc.vector.tensor_copy(qT[hc], ps[:, :NQC])

        # prologue PSUM no longer needed; free its banks for the main loop
        early_ctx.close()
        ps_scores = ctx.enter_context(
            tc.tile_pool(name="ps_scores", bufs=2, space="PSUM")
        )
        ps_et = ctx.enter_context(tc.tile_pool(name="ps_et", bufs=2, space="PSUM"))
        ps_out = ctx.enter_context(tc.tile_pool(name="ps_out", bufs=2, space="PSUM"))

        # ---- main loop ----
        # BASS_AA_REPEAT > 1 duplicates the compute to measure pure device
        # time by scaling (wall = fixed_overhead + repeat * T).
        repeat = int(os.environ.get("BASS_AA_REPEAT", "1"))
        ngroups = 128 // GROUP
        for nb in range(NBLK * repeat):
            nb = nb % NBLK
            scores_ps = ps_scores.tile([128, 512], f32, name="scores_ps")
            # sub-chunks per group: the first chunk of the kernel is small
            # so the ACT stream starts early (pipeline ramp).
            chunks = []
            for g in range(ngroups):
                lo, hi = g * GROUP, (g + 1) * GROUP
                if nb == 0 and g == 0 and repeat == 1:
                    chunks.extend([(lo, lo + 2), (lo + 2, lo + 4), (lo + 4, hi)])
                elif g == ngroups - 1 and repeat == 1:
                    # ramp-down: a small final chunk leaves only a few
                    # matmuls between the last tanh and the softmax.
                    chunks.extend([(lo, lo + 4), (lo + 4, lo + 6), (lo + 6, hi)])
                else:
                    chunks.append((lo, hi))
            if ngroups * GROUP != 128:
                raise AssertionError
            for lo, hi in chunks:
                cn = hi - lo
                w = work_pool.tile([128, 2 * GROUP * M], bf16, name="w")
                for j in range(cn):
                    n = nb * 128 + lo + j
                    for x in range(2):
                        off = (x * cn + j) * M
                        nc.vector.tensor_scalar_add(
                            w[:, off : off + M], kT[x], qT[x][:, n : n + 1]
                        )
                nc.scalar.activation(
                    w[:, : 2 * cn * M], w[:, : 2 * cn * M], AF.Tanh
                )
                for j in range(cn):
                    s = lo + j
                    for x in range(2):
                        off = (x * cn + j) * M
                        nc.tensor.matmul(
                            scores_ps,
                            lhsT=z2[:, x, 128 - s : 256 - s],
                            rhs=w[:, off : off + M],
                            start=(lo == 0 and j == 0 and x == 0),
                            stop=(hi == 128 and j == cn - 1 and x == 1),
                        )

            # softmax over m (no max subtraction: |scores| <~ 4) fused with
            # row sums; normalization deferred to the output rows.
            e_sb = e_pool.tile([128, 512], f32, name="e_sb")
            sums = small.tile([128, 1], f32, name="sums")
            nc.scalar.activation(e_sb, scores_ps, AF.Exp, accum_out=sums)
            recip = small.tile([128, 1], f32, name="recip")
            nc.vector.reciprocal(recip, sums)

            et_ps = ps_et.tile([128, 4, 128], f32, name="et_ps")
            for mc in range(4):
                nc.tensor.transpose(
                    et_ps[:, mc, :], e_sb[:, mc * 128 : (mc + 1) * 128], identity
                )
            last_blk = nb == NBLK - 1 and repeat == 1
            et_sb = et_pool.tile([128, 4, 128], f32, name="et_sb")
            if last_blk:
                nc.scalar.copy(et_sb, et_ps)
            else:
                nc.vector.tensor_copy(et_sb, et_ps)

            ov_ps = ps_out.tile([128, DV], f32, name="ov_ps")
            for mc in range(4):
                nc.tensor.matmul(
                    ov_ps,
                    lhsT=et_sb[:, mc, :],
                    rhs=value_s[:, mc, :],
                    start=(mc == 0),
                    stop=(mc == 3),
                )
            o_sb = out_pool.tile([128, DV], f32, name="o_sb")
            if last_blk:
                nc.scalar.activation(o_sb, ov_ps, AF.Copy, scale=recip)
            else:
                nc.vector.tensor_scalar_mul(o_sb, ov_ps, recip)
            nc.sync.dma_start(out=out_ext[nb * 128 : (nb + 1) * 128, :], in_=o_sb)

    return nc


class _Runner:
    """Persistent jitted SPMD executor (mirrors bass2jax.run_bass_via_pjrt's
    multi-core branch) so repeat calls don't recompile."""

    def __init__(self):
        import jax
        import concourse.mybir as mybir
        from concourse import bass2jax
        from jax.sharding import Mesh, PartitionSpec
        from jax.experimental.shard_map import shard_map

        bass2jax.install_neuronx_cc_hook()
        nc = _build_program()
        self.nc = nc

        partition_name = (
            nc.partition_id_tensor.name if nc.partition_id_tensor else None
        )
        in_names, out_names, out_avals, zero_shapes = [], [], [], []
        for alloc in nc.m.functions[0].allocations:
            if not isinstance(alloc, mybir.MemoryLocationSet):
                continue
            name = alloc.memorylocations[0].name
            if alloc.kind == "ExternalInput":
                if name != partition_name:
                    in_names.append(name)
            elif alloc.kind == "ExternalOutput":
                shape = tuple(alloc.tensor_shape)
                dtype = mybir.dt.np(alloc.dtype)
                out_avals.append(jax.core.ShapedArray(shape, dtype))
                out_names.append(name)
                zero_shapes.append((shape, dtype))
        self.in_names = list(in_names)
        self.out_names = list(out_names)
        self.zero_shapes = zero_shapes
        n_params = len(in_names)
        n_outs = len(out_names)
        all_in_names = in_names + out_names + (
            [partition_name] if partition_name else []
        )

        def _body(*args):
            operands = list(args)
            if partition_name is not None:
                operands.append(bass2jax.partition_id_tensor())
            outs = bass2jax._bass_exec_p.bind(
                *operands,
                out_avals=tuple(out_avals),
                in_names=tuple(all_in_names),
                out_names=tuple(out_names),
                lowering_input_output_aliases=(),
                sim_require_finite=True,
                sim_require_nnan=True,
                nc=nc,
            )
            return tuple(outs)

        devices = jax.devices()[:NCORES]
        mesh = Mesh(np.asarray(devices), ("core",))
        in_specs = (PartitionSpec("core"),) * (n_params + n_outs)
        out_specs = (PartitionSpec("core"),) * n_outs
        self._shardings = [
            jax.sharding.NamedSharding(mesh, PartitionSpec("core"))
        ] * n_params
        self._jit = jax.jit(
            shard_map(
                _body,
                mesh=mesh,
                in_specs=in_specs,
                out_specs=out_specs,
                check_rep=False,
            ),
            donate_argnums=tuple(range(n_params, n_params + n_outs)),
            keep_unused=True,
        )

    def put(self, in_maps):
        """Transfer concatenated inputs to the devices once; returns device
        arrays reusable across run() calls."""
        import jax

        concat_in = [
            np.concatenate([np.asarray(m[name]) for m in in_maps], axis=0)
            for name in self.in_names
        ]
        return jax.block_until_ready(
            [jax.device_put(a, self._shardings[i]) for i, a in enumerate(concat_in)]
        )

    def run(self, dev_in):
        import jax

        concat_zeros = [
            np.zeros((NCORES * s[0], *s[1:]), d) for (s, d) in self.zero_shapes
        ]
        t0 = time.perf_counter()
        outs = jax.block_until_ready(self._jit(*dev_in, *concat_zeros))
        dt = time.perf_counter() - t0
        per_core = [
            {
                name: np.asarray(outs[i]).reshape(NCORES, *self.zero_shapes[i][0])[c]
                for i, name in enumerate(self.out_names)
            }
            for c in range(NCORES)
        ]
        return per_core, dt


def _get_runner():
    global _runner
    if _runner is None:
        _runner = _Runner()
    return _runner


def _shard(query, key, value, W_q, W_k, W_v):
    in_maps = []
    for c in range(NCORES):
        b, half = c // 2, c % 2
        in_maps.append(
            {
                "query": np.ascontiguousarray(
                    query[b, half * NQC : (half + 1) * NQC, :], dtype=np.float32
                ),
                "key": np.ascontiguousarray(key[b], dtype=np.float32),
                "value": np.ascontiguousarray(value[b], dtype=np.float32),
                "W_q": np.ascontiguousarray(W_q, dtype=np.float32),
                "W_k": np.ascontiguousarray(W_k, dtype=np.float32),
                "W_v": np.ascontiguousarray(
                    np.asarray(W_v).reshape(H, 1), dtype=np.float32
                ),
            }
        )
    return in_maps


def _gather(per_core):
    out = np.empty((N, NQ, DV), dtype=np.float32)
    for c in range(NCORES):
        b, half = c // 2, c % 2
        out[b, half * NQC : (half + 1) * NQC, :] = per_core[c]["out"]
    return out


def kernel(query, key, value, W_q, W_k, W_v):
    runner = _get_runner()
    dev_in = runner.put(_shard(np.asarray(query), key, value, W_q, W_k, W_v))
    per_core, _ = runner.run(dev_in)
    return _gather(per_core)


def kernel_timed(query, key, value, W_q, W_k, W_v, iters=5):
    """Returns (output, per-call wall times with device-resident inputs)."""
    runner = _get_runner()
    dev_in = runner.put(_shard(np.asarray(query), key, value, W_q, W_k, W_v))
    times = []
    per_core = None
    for _ in range(iters):
        per_core, dt = runner.run(dev_in)
        times.append(dt)
    return _gather(per_core), times


# revision 33
# speedup vs baseline: 1.1689x; 1.1689x over previous
"""Additive (Bahdanau) attention TRN2 Bass kernel.

Problem (hardcoded shapes):
    query (4, 512, 256), key (4, 512, 256), value (4, 512, 256)
    W_q (256, 256), W_k (256, 256), W_v (256,)
    q = query @ W_q ; k = key @ W_k
    scores[b,n,m] = sum_h W_v[h] * tanh(q[b,n,h] + k[b,m,h])
    out = softmax_m(scores) @ value          -> (4, 512, 256)

Sharding: 8 cores, data-parallel over (batch, query-half):
    core c handles batch b = c // 2, query rows [ (c%2)*256, (c%2)*256+256 ).
Each core sees the full key/value of its batch; outputs are disjoint row
blocks of the full output, so no collectives are needed.

Per-core device algorithm (all fp32):
  1. PE-transpose query/key chunks, project with W_q/W_k -> qT (h,n), kT (h,m)
     laid out with h on partitions.
  2. For each query row n: DVE tensor_scalar add broadcasts q_n[h] over the
     kT tile (h, m); groups of 8 rows share one big ACT tanh instruction.
  3. The h-reduction with W_v runs on PE as a matmul whose stationary is a
     sliding 128-column window of a zero-padded W_v buffer (a delta-column
     matrix), accumulating each row's scores into its own PSUM partition.
  4. Softmax skips the max-subtraction (scores are O(1) by construction):
     ACT exp emits e and its row sums in one instruction, DVE takes
     reciprocals, and the normalization is folded into the output rows.
  5. e is PE-transposed and attn @ value runs as 4 accumulating matmuls.
"""

import os
import time

import numpy as np

N, NQ, M, DQ, DK, DV, H = 4, 512, 512, 256, 256, 256, 256
NCORES = 8
NQC = N * NQ // NCORES  # query rows per core = 256
GROUP = 8  # query rows per ACT tanh instruction
NBLK = NQC // 128  # PSUM score blocks per core

_runner = None


def _build_program():
    from contextlib import ExitStack

    import concourse.bass as bass
    import concourse.mybir as mybir
    import concourse.tile as tile
    from concourse.masks import make_identity
    from concourse.vector_clock import ScopedClock

    f32 = mybir.dt.float32
    bf16 = mybir.dt.bfloat16
    AF = mybir.ActivationFunctionType

    class TileContextChunkedDrain(tile.TileContext):
        """This walrus build rejects instructions carrying more than one sync
        wait. Tile's scheduler freely attaches several, both on scheduled
        instructions and on the exit drain — hoist the extras onto
        single-wait NOPs on the same engine."""

        def _lower_ordered_insts(self, ordered):
            for bb_name, insts in ordered.items():
                new = []
                for inst in insts:
                    si = inst.sync_info
                    if si is not None and si.on_wait and len(si.on_wait) > 1:
                        waits = list(si.on_wait)
                        for wi, w in enumerate(waits[:-1]):
                            nop = mybir.InstNoOp(
                                name=f"{inst.name}-sw{wi}", ins=[], outs=[]
                            )
                            nop.engine = inst.engine
                            nop.sync_info = mybir.SyncInfo(
                                on_wait=[w], on_update=[]
                            )
                            new.append(nop)
                        inst.sync_info = mybir.SyncInfo(
                            on_wait=[waits[-1]], on_update=list(si.on_update)
                        )
                    new.append(inst)
                ordered[bb_name] = new
            return super()._lower_ordered_insts(ordered)

        def _drain_and_barrier(self, tick_clock, wait_clock):
            nc = self.nc
            probe = nc.sync.nop(nofuse=True)
            wait_clock.add_sem_waits(
                probe.ins, ScopedClock({None: tick_clock.global_clock})
            )
            waits = list(probe.ins.sync_info.on_wait)
            probe.ins.sync_info = mybir.SyncInfo(on_wait=waits[:1], on_update=[])
            for w in waits[1:]:
                n2 = nc.sync.nop(nofuse=True)
                n2.ins.sync_info = mybir.SyncInfo(on_wait=[w], on_update=[])
            nc.sync.drain()
            nc.all_engine_barrier()
            popped = nc._tile_sem_poison_stack.pop()
            assert popped is self._sem_poison
            nc.clear_and_free_semaphores(list(self.sems.allocated().values()))
            nc.all_engine_barrier()

    nc = bass.Bass(enable_partition_id=False)
    q_ext = nc.dram_tensor("query", [NQC, DQ], f32, kind="ExternalInput")
    k_ext = nc.dram_tensor("key", [M, DK], f32, kind="ExternalInput")
    v_ext = nc.dram_tensor("value", [M, DV], f32, kind="ExternalInput")
    wq_ext = nc.dram_tensor("W_q", [DQ, H], f32, kind="ExternalInput")
    wk_ext = nc.dram_tensor("W_k", [DK, H], f32, kind="ExternalInput")
    wv_ext = nc.dram_tensor("W_v", [H, 1], f32, kind="ExternalInput")
    out_ext = nc.dram_tensor("out", [NQC, DV], f32, kind="ExternalOutput")

    with TileContextChunkedDrain(nc) as tc, ExitStack() as ctx:
        singles = ctx.enter_context(tc.tile_pool(name="singles", bufs=1))
        loads = ctx.enter_context(tc.tile_pool(name="loads", bufs=2))
        work_pool = ctx.enter_context(tc.tile_pool(name="work", bufs=4))
        e_pool = ctx.enter_context(tc.tile_pool(name="epool", bufs=2))
        et_pool = ctx.enter_context(tc.tile_pool(name="etpool", bufs=2))
        out_pool = ctx.enter_context(tc.tile_pool(name="outpool", bufs=2))
        small = ctx.enter_context(tc.tile_pool(name="small", bufs=4))
        early_ctx = ExitStack()
        ps_early = early_ctx.enter_context(
            tc.tile_pool(name="ps_early", bufs=4, space="PSUM")
        )

        # ---- constants ----
        identity = singles.tile([128, 128], f32)
        make_identity(nc, identity)

        # ---- coalesced input DMAs, critical (key) path first ----
        knat_t = loads.tile([128, 4, DK], f32, name="knat_t")
        nc.sync.dma_start(out=knat_t, in_=k_ext.rearrange("(c p) d -> p c d", p=128))
        wk_s = singles.tile([128, 2, H], f32)
        nc.sync.dma_start(out=wk_s, in_=wk_ext.rearrange("(c p) h -> p c h", p=128))
        qnat_t = loads.tile([128, 2, DQ], f32, name="qnat_t")
        nc.sync.dma_start(out=qnat_t, in_=q_ext.rearrange("(c p) d -> p c d", p=128))
        wq_s = singles.tile([128, 2, H], f32)
        nc.sync.dma_start(out=wq_s, in_=wq_ext.rearrange("(c p) h -> p c h", p=128))
        wv_f = singles.tile([128, 2], f32)
        nc.sync.dma_start(out=wv_f, in_=wv_ext.rearrange("(c p) one -> p (c one)", p=128))
        value_s = singles.tile([128, 4, DV], f32)
        nc.sync.dma_start(out=value_s, in_=v_ext.rearrange("(c p) d -> p c d", p=128))

        # z2[:, x, :] = [0]*128 ++ [W_v[x*128:(x+1)*128]] ++ [0]*127 ; the
        # 128-column window starting at 128-s is W_v * delta(col == s).
        # bf16 so the scores matmul is a single-pass bf16 matmul (fp32
        # matmuls lower to two PE passes).
        z2 = singles.tile([128, 2, 256], bf16)
        nc.vector.memset(z2, 0.0)
        for x in range(2):
            nc.vector.tensor_copy(z2[:, x, 128:129], wv_f[:, x : x + 1])

        qnat = [qnat_t[:, i, :] for i in range(2)]
        knat = [knat_t[:, i, :] for i in range(4)]

        qTd = singles.tile([128, 2, NQC], f32)  # (d_local, dc, n)
        for dc in range(2):
            ps = ps_early.tile([128, 512], f32, name="ps")
            for nck in range(2):
                nc.tensor.transpose(
                    ps[:, nck * 128 : (nck + 1) * 128],
                    qnat[nck][:, dc * 128 : (dc + 1) * 128],
                    identity,
                )
            nc.vector.tensor_copy(qTd[:, dc, :], ps[:, :NQC])

        kTd = singles.tile([128, 2, M], f32)  # (d_local, dc, m)
        for dc in range(2):
            ps = ps_early.tile([128, 512], f32, name="ps")
            for mck in range(4):
                nc.tensor.transpose(
                    ps[:, mck * 128 : (mck + 1) * 128],
                    knat[mck][:, dc * 128 : (dc + 1) * 128],
                    identity,
                )
            nc.scalar.copy(kTd[:, dc, :], ps)

        # ---- projections: qT (h, n), kT (h, m) ----
        # kT is bf16 so the DVE broadcast-adds (bf16 in0 + bf16 out, fp32
        # per-partition scalar) hit the 4x_2p perf mode.
        qT = [singles.tile([128, NQC], f32, name=f"qT{hc}") for hc in range(2)]
        kT = [singles.tile([128, M], bf16, name=f"kT{hc}") for hc in range(2)]
        for hc in range(2):
            ps = ps_early.tile([128, 512], f32, name="ps")
            for dc in range(2):
                nc.tensor.matmul(
                    ps,
                    lhsT=wk_s[:, dc, hc * 128 : (hc + 1) * 128],
                    rhs=kTd[:, dc, :],
                    start=(dc == 0),
                    stop=(dc == 1),
                )
            nc.scalar.copy(kT[hc], ps)
        for hc in range(2):
            ps = ps_early.tile([128, 512], f32, name="ps")
            for dc in range(2):
                nc.tensor.matmul(
                    ps[:, :NQC],
                    lhsT=wq_s[:, dc, hc * 128 : (hc + 1) * 128],
                    rhs=qTd[:, dc, :],
                    start=(dc == 0),
                    stop=(dc == 1),
                )
            nc.vector.tensor_copy(qT[hc], ps[:, :NQC])

        # prologue PSUM no longer needed; free its banks for the main loop
        early_ctx.close()
        ps_scores = ctx.enter_context(
            tc.tile_pool(name="ps_scores", bufs=2, space="PSUM")
        )
        ps_et = ctx.enter_context(tc.tile_pool(name="ps_et", bufs=2, space="PSUM"))
        ps_out = ctx.enter_context(tc.tile_pool(name="ps_out", bufs=2, space="PSUM"))

        # ---- main loop ----
        # BASS_AA_REPEAT > 1 duplicates the compute to measure pure device
        # time by scaling (wall = fixed_overhead + repeat * T).
        repeat = int(os.environ.get("BASS_AA_REPEAT", "1"))
        ngroups = 128 // GROUP
        for nb in range(NBLK * repeat):
            nb = nb % NBLK
            scores_ps = ps_scores.tile([128, 512], f32, name="scores_ps")
            # sub-chunks per group: the first chunk of the kernel is small
            # so the ACT stream starts early (pipeline ramp).
            chunks = []
            for g in range(ngroups):
                lo, hi = g * GROUP, (g + 1) * GROUP
                if nb == 0 and g == 0 and repeat == 1:
                    # ramp-up, first chunk split by h-half: the very first
                    # tanh only needs kT[0]/qT[0] (x=1 projections are
                    # still in flight).
                    chunks.extend(
                        [
                            (lo, lo + 2, (0,)),
                            (lo, lo + 2, (1,)),
                            (lo + 2, lo + 4, (0, 1)),
                            (lo + 4, hi, (0, 1)),
                        ]
                    )
                elif g == ngroups - 1 and repeat == 1:
                    # ramp-down: a small final chunk leaves only a few
                    # matmuls between the last tanh and the softmax.
                    chunks.extend(
                        [
                            (lo, lo + 4, (0, 1)),
                            (lo + 4, lo + 6, (0, 1)),
                            (lo + 6, hi, (0, 1)),
                        ]
                    )
                else:
                    chunks.append((lo, hi, (0, 1)))
            if ngroups * GROUP != 128:
                raise AssertionError
            nmm = sum((hi - lo) * len(xs) for lo, hi, xs in chunks)
            assert nmm == 256
            mm = 0
            for lo, hi, xs in chunks:
                cn = hi - lo
                w = work_pool.tile([128, 2 * GROUP * M], bf16, name="w")
                for j in range(cn):
                    n = nb * 128 + lo + j
                    for xi, x in enumerate(xs):
                        off = (xi * cn + j) * M
                        nc.vector.tensor_scalar_add(
                            w[:, off : off + M], kT[x], qT[x][:, n : n + 1]
                        )
                nc.scalar.activation(
                    w[:, : len(xs) * cn * M], w[:, : len(xs) * cn * M], AF.Tanh
                )
                for j in range(cn):
                    s = lo + j
                    for xi, x in enumerate(xs):
                        off = (xi * cn + j) * M
                        nc.tensor.matmul(
                            scores_ps,
                            lhsT=z2[:, x, 128 - s : 256 - s],
                            rhs=w[:, off : off + M],
                            start=(mm == 0),
                            stop=(mm == nmm - 1),
                        )
                        mm += 1

            # softmax over m (no max subtraction: |scores| <~ 4) fused with
            # row sums; normalization deferred to the output rows.
            e_sb = e_pool.tile([128, 512], f32, name="e_sb")
            sums = small.tile([128, 1], f32, name="sums")
            nc.scalar.activation(e_sb, scores_ps, AF.Exp, accum_out=sums)
            recip = small.tile([128, 1], f32, name="recip")
            nc.vector.reciprocal(recip, sums)

            et_ps = ps_et.tile([128, 4, 128], f32, name="et_ps")
            for mc in range(4):
                nc.tensor.transpose(
                    et_ps[:, mc, :], e_sb[:, mc * 128 : (mc + 1) * 128], identity
                )
            last_blk = nb == NBLK - 1 and repeat == 1
            et_sb = et_pool.tile([128, 4, 128], f32, name="et_sb")
            if last_blk:
                nc.scalar.copy(et_sb, et_ps)
            else:
                nc.vector.tensor_copy(et_sb, et_ps)

            ov_ps = ps_out.tile([128, DV], f32, name="ov_ps")
            for mc in range(4):
                nc.tensor.matmul(
                    ov_ps,
                    lhsT=et_sb[:, mc, :],
                    rhs=value_s[:, mc, :],
                    start=(mc == 0),
                    stop=(mc == 3),
                )
            o_sb = out_pool.tile([128, DV], f32, name="o_sb")
            if last_blk:
                nc.scalar.activation(o_sb, ov_ps, AF.Copy, scale=recip)
            else:
                nc.vector.tensor_scalar_mul(o_sb, ov_ps, recip)
            nc.sync.dma_start(out=out_ext[nb * 128 : (nb + 1) * 128, :], in_=o_sb)

    return nc


class _Runner:
    """Persistent jitted SPMD executor (mirrors bass2jax.run_bass_via_pjrt's
    multi-core branch) so repeat calls don't recompile."""

    def __init__(self):
        import jax
        import concourse.mybir as mybir
        from concourse import bass2jax
        from jax.sharding import Mesh, PartitionSpec
        from jax.experimental.shard_map import shard_map

        bass2jax.install_neuronx_cc_hook()
        nc = _build_program()
        self.nc = nc

        partition_name = (
            nc.partition_id_tensor.name if nc.partition_id_tensor else None
        )
        in_names, out_names, out_avals, zero_shapes = [], [], [], []
        for alloc in nc.m.functions[0].allocations:
            if not isinstance(alloc, mybir.MemoryLocationSet):
                continue
            name = alloc.memorylocations[0].name
            if alloc.kind == "ExternalInput":
                if name != partition_name:
                    in_names.append(name)
            elif alloc.kind == "ExternalOutput":
                shape = tuple(alloc.tensor_shape)
                dtype = mybir.dt.np(alloc.dtype)
                out_avals.append(jax.core.ShapedArray(shape, dtype))
                out_names.append(name)
                zero_shapes.append((shape, dtype))
        self.in_names = list(in_names)
        self.out_names = list(out_names)
        self.zero_shapes = zero_shapes
        n_params = len(in_names)
        n_outs = len(out_names)
        all_in_names = in_names + out_names + (
            [partition_name] if partition_name else []
        )

        def _body(*args):
            operands = list(args)
            if partition_name is not None:
                operands.append(bass2jax.partition_id_tensor())
            outs = bass2jax._bass_exec_p.bind(
                *operands,
                out_avals=tuple(out_avals),
                in_names=tuple(all_in_names),
                out_names=tuple(out_names),
                lowering_input_output_aliases=(),
                sim_require_finite=True,
                sim_require_nnan=True,
                nc=nc,
            )
            return tuple(outs)

        devices = jax.devices()[:NCORES]
        mesh = Mesh(np.asarray(devices), ("core",))
        in_specs = (PartitionSpec("core"),) * (n_params + n_outs)
        out_specs = (PartitionSpec("core"),) * n_outs
        self._shardings = [
            jax.sharding.NamedSharding(mesh, PartitionSpec("core"))
        ] * n_params
        self._jit = jax.jit(
            shard_map(
                _body,
                mesh=mesh,
                in_specs=in_specs,
                out_specs=out_specs,
                check_rep=False,
            ),
            donate_argnums=tuple(range(n_params, n_params + n_outs)),
            keep_unused=True,
        )

    def put(self, in_maps):
        """Transfer concatenated inputs to the devices once; returns device
        arrays reusable across run() calls."""
        import jax

        concat_in = [
            np.concatenate([np.asarray(m[name]) for m in in_maps], axis=0)
            for name in self.in_names
        ]
        return jax.block_until_ready(
            [jax.device_put(a, self._shardings[i]) for i, a in enumerate(concat_in)]
        )

    def run(self, dev_in):
        import jax

        concat_zeros = [
            np.zeros((NCORES * s[0], *s[1:]), d) for (s, d) in self.zero_shapes
        ]
        t0 = time.perf_counter()
        outs = jax.block_until_ready(self._jit(*dev_in, *concat_zeros))
        dt = time.perf_counter() - t0
        per_core = [
            {
                name: np.asarray(outs[i]).reshape(NCORES, *self.zero_shapes[i][0])[c]
                for i, name in enumerate(self.out_names)
            }
            for c in range(NCORES)
        ]
        return per_core, dt


def _get_runner():
    global _runner
    if _runner is None:
        _runner = _Runner()
    return _runner


def _shard(query, key, value, W_q, W_k, W_v):
    in_maps = []
    for c in range(NCORES):
        b, half = c // 2, c % 2
        in_maps.append(
            {
                "query": np.ascontiguousarray(
                    query[b, half * NQC : (half + 1) * NQC, :], dtype=np.float32
                ),
                "key": np.ascontiguousarray(key[b], dtype=np.float32),
                "value": np.ascontiguousarray(value[b], dtype=np.float32),
                "W_q": np.ascontiguousarray(W_q, dtype=np.float32),
                "W_k": np.ascontiguousarray(W_k, dtype=np.float32),
                "W_v": np.ascontiguousarray(
                    np.asarray(W_v).reshape(H, 1), dtype=np.float32
                ),
            }
        )
    return in_maps


def _gather(per_core):
    out = np.empty((N, NQ, DV), dtype=np.float32)
    for c in range(NCORES):
        b, half = c // 2, c % 2
        out[b, half * NQC : (half + 1) * NQC, :] = per_core[c]["out"]
    return out


def kernel(query, key, value, W_q, W_k, W_v):
    runner = _get_runner()
    dev_in = runner.put(_shard(np.asarray(query), key, value, W_q, W_k, W_v))
    per_core, _ = runner.run(dev_in)
    return _gather(per_core)


def kernel_timed(query, key, value, W_q, W_k, W_v, iters=5):
    """Returns (output, per-call wall times with device-resident inputs)."""
    runner = _get_runner()
    dev_in = runner.put(_shard(np.asarray(query), key, value, W_q, W_k, W_v))
    times = []
    per_core = None
    for _ in range(iters):
        per_core, dt = runner.run(dev_in)
        times.append(dt)
    return _gather(per_core), times
